# revision 19
# baseline (speedup 1.0000x reference)
"""EnhancedAdaptiveLoRAPooling fused kernel for 8x Trainium2 NeuronCores.

Strategy (v8 = fp16 streaming, host-side routing prep):
  - hidden_states [8, 4096, 768] is sharded by batch element: core i gets
    x_i [4096, 768], packed on host to fp16 [8, 128, 6, 512]
    (k = 512-token chunk, partition p, hidden chunk c, token t) so every
    DMA partition line is 6 KiB contiguous.  fp16 transport halves HBM
    traffic both ways (rel-err budget is 2e-2; this path measures ~5e-4).
  - Routing is setup math on KB-sized operands (task sims over 16 tasks,
    top-3 + threshold, LoRA pooling, fusion weights), so it is folded on
    the host into one merged rank-16 LoRA — exactly like standard LoRA
    weight-merging — giving Ag [16,768] (scale folded) and Bg [16,768].
    Only ~37 KB of merged operands ship to each core.
  - Device does all O(B*S*H) work: per 512-token chunk
        u  = Ag @ x          (6 chunk matmuls, fp16)
        l  = Bg.T @ u        (6 chunk matmuls, fp16)
        y  = x + l           (DVE adds, fp16 out)
    with u one chunk ahead of l (software pipeline), y streaming out on
    alternating ACT/Pool DMA rings while x streams in on the sync ring.
  - A short PE dummy-matmul warmup keeps the PE clock ramped until the
    first x chunk lands.
  - Memory-bound: ~6.3 MiB in + 6.3 MiB out + ~40 KB consts per core.
"""

import numpy as np

B, S, H = 8, 4096, 768
N_TASKS, R = 16, 8
SCALING = 2.0
TOP_K = 3
NCORES = 8
TPC = (B * S) // NCORES          # tokens per core = 4096
CK = 512                         # token chunk (one PSUM bank wide)
NK = TPC // CK                   # 8 chunks per core
NCH = H // 128                   # 6 hidden chunks
N_WARM_MM = 8                    # PE p-state warmup matmuls

_PROGRAM = None


def _build_program():
    from contextlib import ExitStack

    import concourse.bass as bass  # noqa: F401
    import concourse.tile as tile
    from concourse import bacc, mybir

    f32 = mybir.dt.float32
    f16 = mybir.dt.float16

    nc = bacc.Bacc("TRN2", target_bir_lowering=False, debug=False)

    xin = nc.dram_tensor("xin", [NK, 128, NCH, CK], f16, kind="ExternalInput").ap()
    cblob = nc.dram_tensor("cblob", [128, 48], f32, kind="ExternalInput").ap()
    bblob = nc.dram_tensor("bblob", [16, 384], f32, kind="ExternalInput").ap()
    yout = nc.dram_tensor("yout", [NK, 128, NCH, CK], f16, kind="ExternalOutput").ap()

    with tile.TileContext(nc) as tc:
        with ExitStack() as ctx:
            const = ctx.enter_context(tc.tile_pool(name="const", bufs=1))
            pers = ctx.enter_context(tc.tile_pool(name="pers", bufs=1))
            xp = ctx.enter_context(tc.tile_pool(name="xp", bufs=NK))
            yp = ctx.enter_context(tc.tile_pool(name="yp", bufs=3))
            usb = ctx.enter_context(tc.tile_pool(name="usb", bufs=NK))
            ups = ctx.enter_context(tc.tile_pool(name="ups", bufs=2, space="PSUM"))
            lps = ctx.enter_context(tc.tile_pool(name="lps", bufs=2, space="PSUM"))

            # PE warmup tile (DVE memset, then dummy matmuls ramp the clock)
            wtile = pers.tile([128, CK], f16, name="wtile")
            nc.vector.memset(wtile, 0.0)

            # consts: merged LoRA operands (tiny, land before x0)
            cblob_sb = const.tile([128, 48], f32, name="cblob_sb")
            nc.sync.dma_start(out=cblob_sb, in_=cblob)
            bblob_sb = const.tile([16, 384], f32, name="bblob_sb")
            nc.scalar.dma_start(out=bblob_sb, in_=bblob)
            AgT_sb = cblob_sb.bitcast(f16).rearrange("p (c j) -> p c j", c=6)
            Bg_sb = bblob_sb.bitcast(f16)        # [16, 384] = Bg(768 cols as 6x128)

            xhs = []
            for k in range(NK):
                xh = xp.tile([128, NCH, CK], f16, tag="xh", name=f"xh{k}")
                if k == 0:
                    nc.sync.dma_start(out=xh[:, 0:3, :], in_=xin[k][:, 0:3, :])
                    nc.sync.dma_start(out=xh[:, 3:6, :], in_=xin[k][:, 3:6, :])
                else:
                    nc.sync.dma_start(out=xh, in_=xin[k])
                xhs.append(xh)

            w_ps = ups.tile([16, CK], f32, tag="ups", name="w_ps")
            for _ in range(N_WARM_MM):
                nc.tensor.matmul(w_ps, lhsT=wtile[:, 0:16], rhs=wtile,
                                 start=True, stop=True)

            u_sbs = {}

            def emit_u(*ks):
                # paired bodies share each AgT chunk's LDWEIGHTS
                upss = [ups.tile([16, CK], f32, tag="ups", name="u_ps")
                        for _ in ks]
                for c in range(NCH):
                    for u_ps, k in zip(upss, ks):
                        nc.tensor.matmul(u_ps, lhsT=AgT_sb[:, c, :],
                                         rhs=xhs[k][:, c, :],
                                         start=(c == 0), stop=(c == NCH - 1))
                for u_ps, k in zip(upss, ks):
                    u_sb = usb.tile([16, CK], f16, tag="usb", name="u_sb")
                    nc.scalar.copy(u_sb, u_ps)
                    u_sbs[k] = u_sb

            def emit_lora(k):
                # 3-chunk PSUM tiles: 2 allocations + 2 DVE adds per body
                # (fewer pool rotations -> less SP event-semaphore traffic)
                yt = yp.tile([128, NCH, CK], f16, tag="yt", name="yt")
                for g in range(2):
                    l_ps = lps.tile([128, 3, CK], f32, tag="lora", name="l_ps")
                    for j in range(3):
                        c = g * 3 + j
                        nc.tensor.matmul(l_ps[:, j, :],
                                         lhsT=Bg_sb[:, c * 128:(c + 1) * 128],
                                         rhs=u_sbs[k], start=True, stop=True)
                    nc.vector.tensor_add(yt[:, g * 3:(g + 1) * 3, :],
                                         xhs[k][:, g * 3:(g + 1) * 3, :], l_ps)
                oeng = nc.scalar if k % 2 == 0 else nc.gpsimd
                if k == NK - 1:
                    nc.scalar.dma_start(out=yout[k][:, 0:3, :], in_=yt[:, 0:3, :])
                    nc.gpsimd.dma_start(out=yout[k][:, 3:6, :], in_=yt[:, 3:6, :])
                else:
                    oeng.dma_start(out=yout[k], in_=yt)

            emit_u(0)
            emit_lora(0)
            emit_u(1, 2)
            emit_lora(1)
            emit_u(3, 4)
            emit_lora(2)
            emit_lora(3)
            emit_u(5, 6)
            emit_lora(4)
            emit_lora(5)
            emit_u(7)
            emit_lora(6)
            emit_lora(7)

    nc.compile()
    return nc


def _get_program():
    global _PROGRAM
    if _PROGRAM is None:
        _PROGRAM = _build_program()
    return _PROGRAM


def _chunkpack(a):
    # [C*128, J] -> [128, C*J] so blob[p, c*J+j] = a[c*128+p, j]
    C = a.shape[0] // 128
    return a.reshape(C, 128, -1).transpose(1, 0, 2).reshape(128, -1)


def _routing(cur, la, lb, te, W1, b1, W2, b2, W3, b3, W4, b4, tid):
    """Reference routing math on KB-sized operands -> merged rank-16 LoRA."""
    cn = np.linalg.norm(cur)
    en = np.linalg.norm(te, axis=-1)
    cos = (te @ cur) / np.maximum(en * cn, 1e-8)
    euclid = np.linalg.norm(te - cur[None, :], axis=-1)
    eu_sim = 1.0 / (1.0 + euclid)
    comb = np.concatenate([np.broadcast_to(cur, te.shape), te], axis=-1)
    h = np.maximum(comb @ W1.T + b1, 0.0)
    h = np.maximum(h @ W2.T + b2, 0.0)
    h = np.maximum(h @ W3.T + b3, 0.0)
    nn = 1.0 / (1.0 + np.exp(-(h @ W4.T + b4)))[..., 0]
    sims = 0.4 * cos + 0.3 * eu_sim + 0.3 * nn

    top = np.argsort(-sims, kind="stable")[:TOP_K]
    tv = sims[top]
    w = np.where(tv > 0.0, tv, 0.0)
    tw = float(w.sum())
    safe = tw if tw > 0.0 else 1.0
    pooled_a = np.einsum("k,krh->rh", w, la[top]) / safe      # [R, H]
    pooled_b = np.einsum("k,khr->hr", w, lb[top]) / safe      # [H, R]
    fw = min(0.1 * cn, 0.5)
    fw = fw if tw > 0.0 else 0.0
    c1, c2 = (1.0 - fw) * SCALING, fw * SCALING
    Ag = np.concatenate([c1 * la[tid], c2 * pooled_a], axis=0)    # [16, H]
    Bg = np.concatenate([lb[tid].T, pooled_b.T], axis=0)          # [16, H]
    return Ag.astype(np.float32), Bg.astype(np.float32)


def _make_in_maps(inputs):
    hs = np.ascontiguousarray(np.asarray(inputs["hidden_states"], np.float32))
    cur = np.ascontiguousarray(np.asarray(inputs["task_embedding"], np.float32))
    la = np.ascontiguousarray(np.asarray(inputs["loras_a"], np.float32))
    lb = np.ascontiguousarray(np.asarray(inputs["loras_b"], np.float32))
    te = np.ascontiguousarray(np.asarray(inputs["task_embeds"], np.float32))
    args = [np.asarray(inputs[k], np.float32)
            for k in ("W1", "b1", "W2", "b2", "W3", "b3", "W4", "b4")]
    tid = int(np.asarray(inputs["current_task_id"]))

    Ag, Bg = _routing(cur, la, lb, te, *args, tid)

    def f16pack(a):
        return np.ascontiguousarray(a.astype(np.float16)).view(np.float32)

    cblob = np.ascontiguousarray(
        f16pack(_chunkpack(np.ascontiguousarray(Ag.T))))          # [128, 48]
    assert cblob.shape == (128, 48), cblob.shape
    bblob = np.ascontiguousarray(f16pack(Bg))                     # [16, 384]
    assert bblob.shape == (16, 384), bblob.shape

    rep = {"cblob": cblob, "bblob": bblob}

    x2 = hs.reshape(B * S, H)
    in_maps = []
    for i in range(NCORES):
        shard = x2[i * TPC:(i + 1) * TPC]                         # [TPC, H]
        xpk = shard.reshape(NK, CK, NCH, 128).transpose(0, 3, 2, 1)
        in_maps.append({"xin": np.ascontiguousarray(xpk.astype(np.float16)),
                        **rep})
    return in_maps


def _unpack_core_y(yarr):
    # [NK, 128, NCH, CK] fp16 -> [TPC, H] f32
    return np.ascontiguousarray(
        yarr.transpose(0, 3, 2, 1).astype(np.float32)).reshape(TPC, H)


def kernel(**inputs):
    from concourse.bass_utils import run_bass_kernel_spmd

    nc = _get_program()
    in_maps = _make_in_maps(inputs)
    res = run_bass_kernel_spmd(nc, in_maps, core_ids=list(range(NCORES)))
    out = np.empty((B * S, H), np.float32)
    for i, r in enumerate(res.results):
        out[i * TPC:(i + 1) * TPC] = _unpack_core_y(r["yout"])
    return out.reshape(B, S, H)


# revision 20
# speedup vs baseline: 1.1055x; 1.1055x over previous
"""EnhancedAdaptiveLoRAPooling fused kernel for 8x Trainium2 NeuronCores.

Strategy (v8 = fp16 streaming, host-side routing prep):
  - hidden_states [8, 4096, 768] is sharded by batch element: core i gets
    x_i [4096, 768], packed on host to fp16 [8, 128, 6, 512]
    (k = 512-token chunk, partition p, hidden chunk c, token t) so every
    DMA partition line is 6 KiB contiguous.  fp16 transport halves HBM
    traffic both ways (rel-err budget is 2e-2; this path measures ~5e-4).
  - Routing is setup math on KB-sized operands (task sims over 16 tasks,
    top-3 + threshold, LoRA pooling, fusion weights), so it is folded on
    the host into one merged rank-16 LoRA — exactly like standard LoRA
    weight-merging — giving Ag [16,768] (scale folded) and Bg [16,768].
    Only ~37 KB of merged operands ship to each core.
  - Device does all O(B*S*H) work: per 512-token chunk
        u  = Ag @ x          (6 chunk matmuls, fp16)
        l  = Bg.T @ u        (6 chunk matmuls, fp16)
        y  = x + l           (DVE adds, fp16 out)
    with u one chunk ahead of l (software pipeline), y streaming out on
    alternating ACT/Pool DMA rings while x streams in on the sync ring.
  - A short PE dummy-matmul warmup keeps the PE clock ramped until the
    first x chunk lands.
  - Memory-bound: ~6.3 MiB in + 6.3 MiB out + ~40 KB consts per core.
"""

import numpy as np

B, S, H = 8, 4096, 768
N_TASKS, R = 16, 8
SCALING = 2.0
TOP_K = 3
NCORES = 8
TPC = (B * S) // NCORES          # tokens per core = 4096
CK = 512                         # token chunk (one PSUM bank wide)
NK = TPC // CK                   # 8 chunks per core
NCH = H // 128                   # 6 hidden chunks
N_WARM_MM = 10                   # PE p-state warmup matmuls

_PROGRAM = None


def _build_program():
    from contextlib import ExitStack

    import concourse.bass as bass  # noqa: F401
    import concourse.tile as tile
    from concourse import bacc, mybir

    f32 = mybir.dt.float32
    f16 = mybir.dt.float16

    nc = bacc.Bacc("TRN2", target_bir_lowering=False, debug=False)

    xin = nc.dram_tensor("xin", [NK, 128, NCH, CK], f16, kind="ExternalInput").ap()
    cblob = nc.dram_tensor("cblob", [128, 48], f32, kind="ExternalInput").ap()
    bblob = nc.dram_tensor("bblob", [16, 384], f32, kind="ExternalInput").ap()
    yout = nc.dram_tensor("yout", [NK, 128, NCH, CK], f16, kind="ExternalOutput").ap()

    with tile.TileContext(nc) as tc:
        with ExitStack() as ctx:
            const = ctx.enter_context(tc.tile_pool(name="const", bufs=1))
            pers = ctx.enter_context(tc.tile_pool(name="pers", bufs=1))
            xp = ctx.enter_context(tc.tile_pool(name="xp", bufs=NK))
            yp = ctx.enter_context(tc.tile_pool(name="yp", bufs=3))
            usb = ctx.enter_context(tc.tile_pool(name="usb", bufs=NK))
            ups = ctx.enter_context(tc.tile_pool(name="ups", bufs=2, space="PSUM"))
            lps = ctx.enter_context(tc.tile_pool(name="lps", bufs=2, space="PSUM"))

            # PE warmup tile (DVE memset, then dummy matmuls ramp the clock)
            wtile = pers.tile([128, CK], f16, name="wtile")
            nc.vector.memset(wtile, 0.0)

            # consts: merged LoRA operands (tiny, land before x0)
            cblob_sb = const.tile([128, 48], f32, name="cblob_sb")
            nc.sync.dma_start(out=cblob_sb, in_=cblob)
            bblob_sb = const.tile([16, 384], f32, name="bblob_sb")
            nc.scalar.dma_start(out=bblob_sb, in_=bblob)
            AgT_sb = cblob_sb.bitcast(f16).rearrange("p (c j) -> p c j", c=6)
            Bg_sb = bblob_sb.bitcast(f16)        # [16, 384] = Bg(768 cols as 6x128)

            xhs = []
            for k in range(NK):
                xh = xp.tile([128, NCH, CK], f16, tag="xh", name=f"xh{k}")
                nc.sync.dma_start(out=xh, in_=xin[k])
                xhs.append(xh)

            w_ps = ups.tile([16, CK], f32, tag="ups", name="w_ps")
            for _ in range(N_WARM_MM):
                nc.tensor.matmul(w_ps, lhsT=wtile[:, 0:16], rhs=wtile,
                                 start=True, stop=True)

            u_sbs = {}

            def emit_u(*ks):
                # paired bodies share each AgT chunk's LDWEIGHTS
                upss = [ups.tile([16, CK], f32, tag="ups", name="u_ps")
                        for _ in ks]
                for c in range(NCH):
                    for u_ps, k in zip(upss, ks):
                        nc.tensor.matmul(u_ps, lhsT=AgT_sb[:, c, :],
                                         rhs=xhs[k][:, c, :],
                                         start=(c == 0), stop=(c == NCH - 1))
                for u_ps, k in zip(upss, ks):
                    u_sb = usb.tile([16, CK], f16, tag="usb", name="u_sb")
                    nc.scalar.copy(u_sb, u_ps)
                    u_sbs[k] = u_sb

            def emit_lora(k):
                # 3-chunk PSUM tiles: 2 allocations + 2 DVE adds per body
                # (fewer pool rotations -> less SP event-semaphore traffic)
                yt = yp.tile([128, NCH, CK], f16, tag="yt", name="yt")
                for g in range(2):
                    l_ps = lps.tile([128, 3, CK], f32, tag="lora", name="l_ps")
                    for j in range(3):
                        c = g * 3 + j
                        nc.tensor.matmul(l_ps[:, j, :],
                                         lhsT=Bg_sb[:, c * 128:(c + 1) * 128],
                                         rhs=u_sbs[k], start=True, stop=True)
                    nc.vector.tensor_add(yt[:, g * 3:(g + 1) * 3, :],
                                         xhs[k][:, g * 3:(g + 1) * 3, :], l_ps)
                oeng = nc.scalar if k % 2 == 0 else nc.gpsimd
                oeng.dma_start(out=yout[k], in_=yt)

            emit_u(0)
            emit_lora(0)
            emit_u(1, 2)
            emit_lora(1)
            emit_u(3, 4)
            emit_lora(2)
            emit_lora(3)
            emit_u(5, 6)
            emit_lora(4)
            emit_lora(5)
            emit_u(7)
            emit_lora(6)
            emit_lora(7)

    nc.compile()
    return nc


def _get_program():
    global _PROGRAM
    if _PROGRAM is None:
        _PROGRAM = _build_program()
    return _PROGRAM


def _chunkpack(a):
    # [C*128, J] -> [128, C*J] so blob[p, c*J+j] = a[c*128+p, j]
    C = a.shape[0] // 128
    return a.reshape(C, 128, -1).transpose(1, 0, 2).reshape(128, -1)


def _routing(cur, la, lb, te, W1, b1, W2, b2, W3, b3, W4, b4, tid):
    """Reference routing math on KB-sized operands -> merged rank-16 LoRA."""
    cn = np.linalg.norm(cur)
    en = np.linalg.norm(te, axis=-1)
    cos = (te @ cur) / np.maximum(en * cn, 1e-8)
    euclid = np.linalg.norm(te - cur[None, :], axis=-1)
    eu_sim = 1.0 / (1.0 + euclid)
    comb = np.concatenate([np.broadcast_to(cur, te.shape), te], axis=-1)
    h = np.maximum(comb @ W1.T + b1, 0.0)
    h = np.maximum(h @ W2.T + b2, 0.0)
    h = np.maximum(h @ W3.T + b3, 0.0)
    nn = 1.0 / (1.0 + np.exp(-(h @ W4.T + b4)))[..., 0]
    sims = 0.4 * cos + 0.3 * eu_sim + 0.3 * nn

    top = np.argsort(-sims, kind="stable")[:TOP_K]
    tv = sims[top]
    w = np.where(tv > 0.0, tv, 0.0)
    tw = float(w.sum())
    safe = tw if tw > 0.0 else 1.0
    pooled_a = np.einsum("k,krh->rh", w, la[top]) / safe      # [R, H]
    pooled_b = np.einsum("k,khr->hr", w, lb[top]) / safe      # [H, R]
    fw = min(0.1 * cn, 0.5)
    fw = fw if tw > 0.0 else 0.0
    c1, c2 = (1.0 - fw) * SCALING, fw * SCALING
    Ag = np.concatenate([c1 * la[tid], c2 * pooled_a], axis=0)    # [16, H]
    Bg = np.concatenate([lb[tid].T, pooled_b.T], axis=0)          # [16, H]
    return Ag.astype(np.float32), Bg.astype(np.float32)


def _make_in_maps(inputs):
    hs = np.ascontiguousarray(np.asarray(inputs["hidden_states"], np.float32))
    cur = np.ascontiguousarray(np.asarray(inputs["task_embedding"], np.float32))
    la = np.ascontiguousarray(np.asarray(inputs["loras_a"], np.float32))
    lb = np.ascontiguousarray(np.asarray(inputs["loras_b"], np.float32))
    te = np.ascontiguousarray(np.asarray(inputs["task_embeds"], np.float32))
    args = [np.asarray(inputs[k], np.float32)
            for k in ("W1", "b1", "W2", "b2", "W3", "b3", "W4", "b4")]
    tid = int(np.asarray(inputs["current_task_id"]))

    Ag, Bg = _routing(cur, la, lb, te, *args, tid)

    def f16pack(a):
        return np.ascontiguousarray(a.astype(np.float16)).view(np.float32)

    cblob = np.ascontiguousarray(
        f16pack(_chunkpack(np.ascontiguousarray(Ag.T))))          # [128, 48]
    assert cblob.shape == (128, 48), cblob.shape
    bblob = np.ascontiguousarray(f16pack(Bg))                     # [16, 384]
    assert bblob.shape == (16, 384), bblob.shape

    rep = {"cblob": cblob, "bblob": bblob}

    x2 = hs.reshape(B * S, H)
    in_maps = []
    for i in range(NCORES):
        shard = x2[i * TPC:(i + 1) * TPC]                         # [TPC, H]
        xpk = shard.reshape(NK, CK, NCH, 128).transpose(0, 3, 2, 1)
        in_maps.append({"xin": np.ascontiguousarray(xpk.astype(np.float16)),
                        **rep})
    return in_maps


def _unpack_core_y(yarr):
    # [NK, 128, NCH, CK] fp16 -> [TPC, H] f32
    return np.ascontiguousarray(
        yarr.transpose(0, 3, 2, 1).astype(np.float32)).reshape(TPC, H)


def kernel(**inputs):
    from concourse.bass_utils import run_bass_kernel_spmd

    nc = _get_program()
    in_maps = _make_in_maps(inputs)
    res = run_bass_kernel_spmd(nc, in_maps, core_ids=list(range(NCORES)))
    out = np.empty((B * S, H), np.float32)
    for i, r in enumerate(res.results):
        out[i * TPC:(i + 1) * TPC] = _unpack_core_y(r["yout"])
    return out.reshape(B, S, H)


# revision 21
# speedup vs baseline: 1.1178x; 1.0111x over previous
"""EnhancedAdaptiveLoRAPooling fused kernel for 8x Trainium2 NeuronCores.

Strategy (v8 = fp16 streaming, host-side routing prep):
  - hidden_states [8, 4096, 768] is sharded by batch element: core i gets
    x_i [4096, 768], packed on host to fp16 [8, 128, 6, 512]
    (k = 512-token chunk, partition p, hidden chunk c, token t) so every
    DMA partition line is 6 KiB contiguous.  fp16 transport halves HBM
    traffic both ways (rel-err budget is 2e-2; this path measures ~5e-4).
  - Routing is setup math on KB-sized operands (task sims over 16 tasks,
    top-3 + threshold, LoRA pooling, fusion weights), so it is folded on
    the host into one merged rank-16 LoRA — exactly like standard LoRA
    weight-merging — giving Ag [16,768] (scale folded) and Bg [16,768].
    Only ~37 KB of merged operands ship to each core.
  - Device does all O(B*S*H) work: per 512-token chunk
        u  = Ag @ x          (6 chunk matmuls, fp16)
        l  = Bg.T @ u        (6 chunk matmuls, fp16)
        y  = x + l           (DVE adds, fp16 out)
    with u one chunk ahead of l (software pipeline), y streaming out on
    alternating ACT/Pool DMA rings while x streams in on the sync ring.
  - A short PE dummy-matmul warmup keeps the PE clock ramped until the
    first x chunk lands.
  - Memory-bound: ~6.3 MiB in + 6.3 MiB out + ~40 KB consts per core.
"""

import numpy as np

B, S, H = 8, 4096, 768
N_TASKS, R = 16, 8
SCALING = 2.0
TOP_K = 3
NCORES = 8
TPC = (B * S) // NCORES          # tokens per core = 4096
CK = 512                         # token chunk (one PSUM bank wide)
NK = TPC // CK                   # 8 chunks per core
NCH = H // 128                   # 6 hidden chunks
N_WARM_MM = 6                    # PE p-state warmup matmuls

_PROGRAM = None


def _build_program():
    from contextlib import ExitStack

    import concourse.bass as bass  # noqa: F401
    import concourse.tile as tile
    from concourse import bacc, mybir

    f32 = mybir.dt.float32
    f16 = mybir.dt.float16

    nc = bacc.Bacc("TRN2", target_bir_lowering=False, debug=False)

    xin = nc.dram_tensor("xin", [NK, 128, NCH, CK], f16, kind="ExternalInput").ap()
    cblob = nc.dram_tensor("cblob", [128, 48], f32, kind="ExternalInput").ap()
    bblob = nc.dram_tensor("bblob", [16, 384], f32, kind="ExternalInput").ap()
    yout = nc.dram_tensor("yout", [NK, 128, NCH, CK], f16, kind="ExternalOutput").ap()

    with tile.TileContext(nc) as tc:
        with ExitStack() as ctx:
            const = ctx.enter_context(tc.tile_pool(name="const", bufs=1))
            pers = ctx.enter_context(tc.tile_pool(name="pers", bufs=1))
            xp = ctx.enter_context(tc.tile_pool(name="xp", bufs=NK))
            yp = ctx.enter_context(tc.tile_pool(name="yp", bufs=3))
            usb = ctx.enter_context(tc.tile_pool(name="usb", bufs=NK))
            ups = ctx.enter_context(tc.tile_pool(name="ups", bufs=2, space="PSUM"))
            lps = ctx.enter_context(tc.tile_pool(name="lps", bufs=2, space="PSUM"))

            # PE warmup tile (DVE memset, then dummy matmuls ramp the clock)
            wtile = pers.tile([128, CK], f16, name="wtile")
            nc.vector.memset(wtile, 0.0)

            # consts: merged LoRA operands (tiny, land before x0)
            cblob_sb = const.tile([128, 48], f32, name="cblob_sb")
            nc.sync.dma_start(out=cblob_sb, in_=cblob)
            bblob_sb = const.tile([16, 384], f32, name="bblob_sb")
            nc.scalar.dma_start(out=bblob_sb, in_=bblob)
            AgT_sb = cblob_sb.bitcast(f16).rearrange("p (c j) -> p c j", c=6)
            Bg_sb = bblob_sb.bitcast(f16)        # [16, 384] = Bg(768 cols as 6x128)

            xhs = []
            for k in range(NK):
                xh = xp.tile([128, NCH, CK], f16, tag="xh", name=f"xh{k}")
                nc.sync.dma_start(out=xh, in_=xin[k])
                xhs.append(xh)

            w_ps = ups.tile([16, CK], f32, tag="ups", name="w_ps")
            for _ in range(N_WARM_MM):
                nc.tensor.matmul(w_ps, lhsT=wtile[:, 0:16], rhs=wtile,
                                 start=True, stop=True)

            u_sbs = {}

            def emit_u(*ks):
                # paired bodies share each AgT chunk's LDWEIGHTS
                upss = [ups.tile([16, CK], f32, tag="ups", name="u_ps")
                        for _ in ks]
                for c in range(NCH):
                    for u_ps, k in zip(upss, ks):
                        nc.tensor.matmul(u_ps, lhsT=AgT_sb[:, c, :],
                                         rhs=xhs[k][:, c, :],
                                         start=(c == 0), stop=(c == NCH - 1))
                for u_ps, k in zip(upss, ks):
                    u_sb = usb.tile([16, CK], f16, tag="usb", name="u_sb")
                    nc.scalar.copy(u_sb, u_ps)
                    u_sbs[k] = u_sb

            def emit_lora_group(k, g, yt):
                # 3-chunk PSUM tile: 1 allocation + 1 DVE add per group
                # (fewer pool rotations -> less SP event-semaphore traffic)
                l_ps = lps.tile([128, 3, CK], f32, tag="lora", name="l_ps")
                for j in range(3):
                    c = g * 3 + j
                    nc.tensor.matmul(l_ps[:, j, :],
                                     lhsT=Bg_sb[:, c * 128:(c + 1) * 128],
                                     rhs=u_sbs[k], start=True, stop=True)
                nc.vector.tensor_add(yt[:, g * 3:(g + 1) * 3, :],
                                     xhs[k][:, g * 3:(g + 1) * 3, :], l_ps)

            def emit_y_dma(k, yt):
                oeng = nc.scalar if k % 2 == 0 else nc.gpsimd
                oeng.dma_start(out=yout[k], in_=yt)

            def emit_lora(k):
                yt = yp.tile([128, NCH, CK], f16, tag="yt", name="yt")
                emit_lora_group(k, 0, yt)
                emit_lora_group(k, 1, yt)
                emit_y_dma(k, yt)

            emit_u(0)
            emit_lora(0)
            emit_u(1, 2)
            emit_lora(1)
            emit_u(3, 4)
            emit_lora(2)
            emit_lora(3)
            emit_u(5, 6)
            emit_lora(4)
            emit_lora(5)
            # interleave the last two bodies so the PE never idles while the
            # DVE drains a group (keeps the PE clock ramped through the tail)
            yt6 = yp.tile([128, NCH, CK], f16, tag="yt", name="yt6")
            yt7 = yp.tile([128, NCH, CK], f16, tag="yt", name="yt7")
            emit_lora_group(6, 0, yt6)
            emit_u(7)
            emit_lora_group(6, 1, yt6)
            emit_y_dma(6, yt6)
            emit_lora_group(7, 0, yt7)
            emit_lora_group(7, 1, yt7)
            emit_y_dma(7, yt7)

    nc.compile()
    return nc


def _get_program():
    global _PROGRAM
    if _PROGRAM is None:
        _PROGRAM = _build_program()
    return _PROGRAM


def _chunkpack(a):
    # [C*128, J] -> [128, C*J] so blob[p, c*J+j] = a[c*128+p, j]
    C = a.shape[0] // 128
    return a.reshape(C, 128, -1).transpose(1, 0, 2).reshape(128, -1)


def _routing(cur, la, lb, te, W1, b1, W2, b2, W3, b3, W4, b4, tid):
    """Reference routing math on KB-sized operands -> merged rank-16 LoRA."""
    cn = np.linalg.norm(cur)
    en = np.linalg.norm(te, axis=-1)
    cos = (te @ cur) / np.maximum(en * cn, 1e-8)
    euclid = np.linalg.norm(te - cur[None, :], axis=-1)
    eu_sim = 1.0 / (1.0 + euclid)
    comb = np.concatenate([np.broadcast_to(cur, te.shape), te], axis=-1)
    h = np.maximum(comb @ W1.T + b1, 0.0)
    h = np.maximum(h @ W2.T + b2, 0.0)
    h = np.maximum(h @ W3.T + b3, 0.0)
    nn = 1.0 / (1.0 + np.exp(-(h @ W4.T + b4)))[..., 0]
    sims = 0.4 * cos + 0.3 * eu_sim + 0.3 * nn

    top = np.argsort(-sims, kind="stable")[:TOP_K]
    tv = sims[top]
    w = np.where(tv > 0.0, tv, 0.0)
    tw = float(w.sum())
    safe = tw if tw > 0.0 else 1.0
    pooled_a = np.einsum("k,krh->rh", w, la[top]) / safe      # [R, H]
    pooled_b = np.einsum("k,khr->hr", w, lb[top]) / safe      # [H, R]
    fw = min(0.1 * cn, 0.5)
    fw = fw if tw > 0.0 else 0.0
    c1, c2 = (1.0 - fw) * SCALING, fw * SCALING
    Ag = np.concatenate([c1 * la[tid], c2 * pooled_a], axis=0)    # [16, H]
    Bg = np.concatenate([lb[tid].T, pooled_b.T], axis=0)          # [16, H]
    return Ag.astype(np.float32), Bg.astype(np.float32)


def _make_in_maps(inputs):
    hs = np.ascontiguousarray(np.asarray(inputs["hidden_states"], np.float32))
    cur = np.ascontiguousarray(np.asarray(inputs["task_embedding"], np.float32))
    la = np.ascontiguousarray(np.asarray(inputs["loras_a"], np.float32))
    lb = np.ascontiguousarray(np.asarray(inputs["loras_b"], np.float32))
    te = np.ascontiguousarray(np.asarray(inputs["task_embeds"], np.float32))
    args = [np.asarray(inputs[k], np.float32)
            for k in ("W1", "b1", "W2", "b2", "W3", "b3", "W4", "b4")]
    tid = int(np.asarray(inputs["current_task_id"]))

    Ag, Bg = _routing(cur, la, lb, te, *args, tid)

    def f16pack(a):
        return np.ascontiguousarray(a.astype(np.float16)).view(np.float32)

    cblob = np.ascontiguousarray(
        f16pack(_chunkpack(np.ascontiguousarray(Ag.T))))          # [128, 48]
    assert cblob.shape == (128, 48), cblob.shape
    bblob = np.ascontiguousarray(f16pack(Bg))                     # [16, 384]
    assert bblob.shape == (16, 384), bblob.shape

    rep = {"cblob": cblob, "bblob": bblob}

    x2 = hs.reshape(B * S, H)
    in_maps = []
    for i in range(NCORES):
        shard = x2[i * TPC:(i + 1) * TPC]                         # [TPC, H]
        xpk = shard.reshape(NK, CK, NCH, 128).transpose(0, 3, 2, 1)
        in_maps.append({"xin": np.ascontiguousarray(xpk.astype(np.float16)),
                        **rep})
    return in_maps


def _unpack_core_y(yarr):
    # [NK, 128, NCH, CK] fp16 -> [TPC, H] f32
    return np.ascontiguousarray(
        yarr.transpose(0, 3, 2, 1).astype(np.float32)).reshape(TPC, H)


def kernel(**inputs):
    from concourse.bass_utils import run_bass_kernel_spmd

    nc = _get_program()
    in_maps = _make_in_maps(inputs)
    res = run_bass_kernel_spmd(nc, in_maps, core_ids=list(range(NCORES)))
    out = np.empty((B * S, H), np.float32)
    for i, r in enumerate(res.results):
        out[i * TPC:(i + 1) * TPC] = _unpack_core_y(r["yout"])
    return out.reshape(B, S, H)


# revision 25
# speedup vs baseline: 1.2059x; 1.0788x over previous
"""EnhancedAdaptiveLoRAPooling fused kernel for 8x Trainium2 NeuronCores.

Strategy (v8 = fp16 streaming, host-side routing prep):
  - hidden_states [8, 4096, 768] is sharded by batch element: core i gets
    x_i [4096, 768], packed on host to fp16 [8, 128, 6, 512]
    (k = 512-token chunk, partition p, hidden chunk c, token t) so every
    DMA partition line is 6 KiB contiguous.  fp16 transport halves HBM
    traffic both ways (rel-err budget is 2e-2; this path measures ~5e-4).
  - Routing is setup math on KB-sized operands (task sims over 16 tasks,
    top-3 + threshold, LoRA pooling, fusion weights), so it is folded on
    the host into one merged rank-16 LoRA — exactly like standard LoRA
    weight-merging — giving Ag [16,768] (scale folded) and Bg [16,768].
    Only ~37 KB of merged operands ship to each core.
  - Device does all O(B*S*H) work: per 512-token chunk
        u  = Ag @ x          (6 chunk matmuls, fp16)
        l  = Bg.T @ u        (6 chunk matmuls, fp16)
        y  = x + l           (DVE adds, fp16 out)
    with u one chunk ahead of l (software pipeline), y streaming out on
    alternating ACT/Pool DMA rings while x streams in on the sync ring.
  - A short PE dummy-matmul warmup keeps the PE clock ramped until the
    first x chunk lands.
  - Memory-bound: ~6.3 MiB in + 6.3 MiB out + ~40 KB consts per core.
"""

import numpy as np

B, S, H = 8, 4096, 768
N_TASKS, R = 16, 8
SCALING = 2.0
TOP_K = 3
NCORES = 8
TPC = (B * S) // NCORES          # tokens per core = 4096
CK = 512                         # token chunk (one PSUM bank wide)
NK = TPC // CK                   # 8 chunks per core
NCH = H // 128                   # 6 hidden chunks
N_WARM_MM = 6                    # PE p-state warmup matmuls

_PROGRAM = None


def _build_program():
    from contextlib import ExitStack

    import concourse.bass as bass  # noqa: F401
    import concourse.tile as tile
    from concourse import bacc, mybir

    f32 = mybir.dt.float32
    f16 = mybir.dt.float16

    nc = bacc.Bacc("TRN2", target_bir_lowering=False, debug=False)

    xin = nc.dram_tensor("xin", [NK, 128, NCH, CK], f16, kind="ExternalInput").ap()
    cblob = nc.dram_tensor("cblob", [128, 48], f32, kind="ExternalInput").ap()
    bblob = nc.dram_tensor("bblob", [16, 384], f32, kind="ExternalInput").ap()
    yout = nc.dram_tensor("yout", [NK, 128, NCH, CK], f16, kind="ExternalOutput").ap()

    with tile.TileContext(nc) as tc:
        with ExitStack() as ctx:
            const = ctx.enter_context(tc.tile_pool(name="const", bufs=1))
            pers = ctx.enter_context(tc.tile_pool(name="pers", bufs=1))
            xp = ctx.enter_context(tc.tile_pool(name="xp", bufs=NK))
            yp = ctx.enter_context(tc.tile_pool(name="yp", bufs=4))
            usb = ctx.enter_context(tc.tile_pool(name="usb", bufs=NK))
            lsb = ctx.enter_context(tc.tile_pool(name="lsb", bufs=3))
            ups = ctx.enter_context(tc.tile_pool(name="ups", bufs=2, space="PSUM"))
            lps = ctx.enter_context(tc.tile_pool(name="lps", bufs=2, space="PSUM"))

            # PE warmup tile (DVE memset, then dummy matmuls ramp the clock)
            wtile = pers.tile([128, CK], f16, name="wtile")
            nc.vector.memset(wtile, 0.0)

            # consts: merged LoRA operands (tiny, land before x0)
            cblob_sb = const.tile([128, 48], f32, name="cblob_sb")
            nc.sync.dma_start(out=cblob_sb, in_=cblob)
            bblob_sb = const.tile([16, 384], f32, name="bblob_sb")
            nc.scalar.dma_start(out=bblob_sb, in_=bblob)
            AgT_sb = cblob_sb.bitcast(f16).rearrange("p (c j) -> p c j", c=6)
            Bg_sb = bblob_sb.bitcast(f16)        # [16, 384] = Bg(768 cols as 6x128)

            xhs = []
            for k in range(NK):
                xh = xp.tile([128, NCH, CK], f16, tag="xh", name=f"xh{k}")
                nc.sync.dma_start(out=xh, in_=xin[k])
                xhs.append(xh)

            w_ps = ups.tile([16, CK], f32, tag="ups", name="w_ps")
            for _ in range(N_WARM_MM):
                nc.tensor.matmul(w_ps, lhsT=wtile[:, 0:16], rhs=wtile,
                                 start=True, stop=True)

            u_sbs = {}

            def emit_u(*ks):
                # paired bodies share each AgT chunk's LDWEIGHTS
                upss = [ups.tile([16, CK], f32, tag="ups", name="u_ps")
                        for _ in ks]
                for c in range(NCH):
                    for u_ps, k in zip(upss, ks):
                        nc.tensor.matmul(u_ps, lhsT=AgT_sb[:, c, :],
                                         rhs=xhs[k][:, c, :],
                                         start=(c == 0), stop=(c == NCH - 1))
                for u_ps, k in zip(upss, ks):
                    u_sb = usb.tile([16, CK], f16, tag="usb", name="u_sb")
                    nc.scalar.copy(u_sb, u_ps)
                    u_sbs[k] = u_sb

            def emit_lora_group(k, g, yt, pool_add=False):
                # 3-chunk PSUM tile: 1 allocation + 1 add per group
                # (fewer pool rotations -> less SP event-semaphore traffic)
                l_ps = lps.tile([128, 3, CK], f32, tag="lora", name="l_ps")
                for j in range(3):
                    c = g * 3 + j
                    nc.tensor.matmul(l_ps[:, j, :],
                                     lhsT=Bg_sb[:, c * 128:(c + 1) * 128],
                                     rhs=u_sbs[k], start=True, stop=True)
                if pool_add:
                    # DVE is the saturated engine; route this group through
                    # ACT (PSUM drain) + GpSimd (SBUF-only add, idle engine)
                    l_sb = lsb.tile([128, 3, CK], f16, tag="lsb", name="l_sb")
                    nc.scalar.copy(l_sb, l_ps)
                    nc.gpsimd.tensor_add(yt[:, g * 3:(g + 1) * 3, :],
                                         xhs[k][:, g * 3:(g + 1) * 3, :], l_sb)
                else:
                    nc.vector.tensor_add(yt[:, g * 3:(g + 1) * 3, :],
                                         xhs[k][:, g * 3:(g + 1) * 3, :], l_ps)

            def emit_y_dma(k, yt):
                oeng = nc.scalar if k % 2 == 0 else nc.gpsimd
                oeng.dma_start(out=yout[k], in_=yt)

            def emit_lora(k):
                yt = yp.tile([128, NCH, CK], f16, tag="yt", name="yt")
                emit_lora_group(k, 0, yt)
                emit_lora_group(k, 1, yt, pool_add=(k <= 6))
                emit_y_dma(k, yt)

            emit_u(0)
            emit_lora(0)
            emit_u(1, 2)
            emit_lora(1)
            emit_u(3, 4)
            emit_lora(2)
            emit_lora(3)
            emit_u(5, 6)
            emit_lora(4)
            emit_lora(5)
            # interleave the last two bodies so the PE never idles while the
            # DVE drains a group (keeps the PE clock ramped through the tail)
            yt6 = yp.tile([128, NCH, CK], f16, tag="yt", name="yt6")
            yt7 = yp.tile([128, NCH, CK], f16, tag="yt", name="yt7")
            emit_lora_group(6, 0, yt6)
            emit_u(7)
            emit_lora_group(6, 1, yt6)
            emit_y_dma(6, yt6)
            emit_lora_group(7, 0, yt7)
            emit_lora_group(7, 1, yt7)
            emit_y_dma(7, yt7)

    nc.compile()
    return nc


def _get_program():
    global _PROGRAM
    if _PROGRAM is None:
        _PROGRAM = _build_program()
    return _PROGRAM


def _chunkpack(a):
    # [C*128, J] -> [128, C*J] so blob[p, c*J+j] = a[c*128+p, j]
    C = a.shape[0] // 128
    return a.reshape(C, 128, -1).transpose(1, 0, 2).reshape(128, -1)


def _routing(cur, la, lb, te, W1, b1, W2, b2, W3, b3, W4, b4, tid):
    """Reference routing math on KB-sized operands -> merged rank-16 LoRA."""
    cn = np.linalg.norm(cur)
    en = np.linalg.norm(te, axis=-1)
    cos = (te @ cur) / np.maximum(en * cn, 1e-8)
    euclid = np.linalg.norm(te - cur[None, :], axis=-1)
    eu_sim = 1.0 / (1.0 + euclid)
    comb = np.concatenate([np.broadcast_to(cur, te.shape), te], axis=-1)
    h = np.maximum(comb @ W1.T + b1, 0.0)
    h = np.maximum(h @ W2.T + b2, 0.0)
    h = np.maximum(h @ W3.T + b3, 0.0)
    nn = 1.0 / (1.0 + np.exp(-(h @ W4.T + b4)))[..., 0]
    sims = 0.4 * cos + 0.3 * eu_sim + 0.3 * nn

    top = np.argsort(-sims, kind="stable")[:TOP_K]
    tv = sims[top]
    w = np.where(tv > 0.0, tv, 0.0)
    tw = float(w.sum())
    safe = tw if tw > 0.0 else 1.0
    pooled_a = np.einsum("k,krh->rh", w, la[top]) / safe      # [R, H]
    pooled_b = np.einsum("k,khr->hr", w, lb[top]) / safe      # [H, R]
    fw = min(0.1 * cn, 0.5)
    fw = fw if tw > 0.0 else 0.0
    c1, c2 = (1.0 - fw) * SCALING, fw * SCALING
    Ag = np.concatenate([c1 * la[tid], c2 * pooled_a], axis=0)    # [16, H]
    Bg = np.concatenate([lb[tid].T, pooled_b.T], axis=0)          # [16, H]
    return Ag.astype(np.float32), Bg.astype(np.float32)


def _make_in_maps(inputs):
    hs = np.ascontiguousarray(np.asarray(inputs["hidden_states"], np.float32))
    cur = np.ascontiguousarray(np.asarray(inputs["task_embedding"], np.float32))
    la = np.ascontiguousarray(np.asarray(inputs["loras_a"], np.float32))
    lb = np.ascontiguousarray(np.asarray(inputs["loras_b"], np.float32))
    te = np.ascontiguousarray(np.asarray(inputs["task_embeds"], np.float32))
    args = [np.asarray(inputs[k], np.float32)
            for k in ("W1", "b1", "W2", "b2", "W3", "b3", "W4", "b4")]
    tid = int(np.asarray(inputs["current_task_id"]))

    Ag, Bg = _routing(cur, la, lb, te, *args, tid)

    def f16pack(a):
        return np.ascontiguousarray(a.astype(np.float16)).view(np.float32)

    cblob = np.ascontiguousarray(
        f16pack(_chunkpack(np.ascontiguousarray(Ag.T))))          # [128, 48]
    assert cblob.shape == (128, 48), cblob.shape
    bblob = np.ascontiguousarray(f16pack(Bg))                     # [16, 384]
    assert bblob.shape == (16, 384), bblob.shape

    rep = {"cblob": cblob, "bblob": bblob}

    x2 = hs.reshape(B * S, H)
    in_maps = []
    for i in range(NCORES):
        shard = x2[i * TPC:(i + 1) * TPC]                         # [TPC, H]
        xpk = shard.reshape(NK, CK, NCH, 128).transpose(0, 3, 2, 1)
        in_maps.append({"xin": np.ascontiguousarray(xpk.astype(np.float16)),
                        **rep})
    return in_maps


def _unpack_core_y(yarr):
    # [NK, 128, NCH, CK] fp16 -> [TPC, H] f32
    return np.ascontiguousarray(
        yarr.transpose(0, 3, 2, 1).astype(np.float32)).reshape(TPC, H)


def kernel(**inputs):
    from concourse.bass_utils import run_bass_kernel_spmd

    nc = _get_program()
    in_maps = _make_in_maps(inputs)
    res = run_bass_kernel_spmd(nc, in_maps, core_ids=list(range(NCORES)))
    out = np.empty((B * S, H), np.float32)
    for i, r in enumerate(res.results):
        out[i * TPC:(i + 1) * TPC] = _unpack_core_y(r["yout"])
    return out.reshape(B, S, H)
